# revision 49
# baseline (speedup 1.0000x reference)
"""Trainium2 Bass kernel for nn_EstLossSepEmb (contrastive eval loss_fn).

Strategy (data-parallel over the batch dim, 8 cores, 1024 rows each):
  - Host prep (layout/dtype only): normalize caption_emb rows and cast to
    fp8e4; cast the three query tensors to fp8e4 and the four loss-side
    tensors to bf16; build per-core SBUF-image layouts ([128, 2, N]:
    k-chunk pairs per partition) with caption_emb rolled so each core's
    own 1024 text rows come first; precompute the per-row diagonal bias
    -(d_i+TOL) from the same fp8 bytes (exact in fp64; the PSUM-fp32 sim
    value of the diagonal deviates by ~1e-6 << TOL).
  - Device (all the FLOPs):
      * 3 big sims as fp8e4 DoubleRow matmuls [1024,256]x[256,8192]:
        K=256 contracted in ONE pass at 2x bf16 rate, 512 out-cols per
        matmul (s3d3 ISA max), weights stationary per (x, m-block) with
        duplicate LDWEIGHTS removed by a custom post-schedule pass.
        fp8 sim noise ~0.004 cos vs measured min decision margin 0.030.
      * per row decide "argmax == i" as (d+TOL >= rowmax) AND
        count(sim > d+TOL) == 0. PSUM groups of 1536/1536/1024 columns
        rotate through three single-buffered slots (3+3+2 banks =
        pipeline depth 3); each group is consumed whole by ACT
        Sign(bias=-(d+TOL))+accum (count) or DVE reduce_max (rowmax) --
        the only two PSUM-capable consumers -- strictly alternating so
        both run concurrently.
      * rowwise-cos loss ingredients (dots + sumsqs of bf16 inputs):
        GPSIMD elementwise products (otherwise-idle engine) + per-m-block
        ones-matmul column sums into the 1024-slot, in [128, MB] layout.
      * emission order: (x, m)-major with a chunk-arrival prologue for
        the first four blocks; loss colsums every other block; per-x
        slot folds + output DMAs as soon as that x finishes.
  - Host combine: means / cos / counts -> the 9-vector output.
"""

import os

import numpy as np

BB = 8192
DIM = 256
NCORES = 8
RPC = BB // NCORES  # rows per core = 1024
MB = RPC // 128  # m blocks per core = 8
KCH = DIM // 128  # 2 k-chunks
# uniform 2048-column PSUM groups: two rotating single-buffer tiles
# (4+4 banks, pipeline depth 2) with minimal per-group overhead
GSIZES = [2048, 2048, 2048, 2048]
GOFF = [0, 2048, 4096, 6144]
NG = len(GSIZES)  # 4 groups per (x, m)
MMN = int(os.environ.get("K_MMN", "512"))  # out cols per DoubleRow matmul
TOL = 1e-4
EPS = 1e-8
NEG_INF = -3.0e38

# consumer-engine shares for the 96 (x, m, g) sim groups (ACT Sign+accum
# count vs DVE reduce_max; GPSIMD cannot access PSUM on TRN2)
W_ACT = float(os.environ.get("K_WA", "0.49"))
W_DVE = float(os.environ.get("K_WD", "0.51"))

Q_NAMES = ["q_dot_vf", "q_ss_v", "q_ss_gv", "q_dot_tc", "q_ss_tp", "q_ss_ce"]
X_NAMES = ["v", "gv", "nv"]


def _assignments():
    """Strict per-group engine alternation with per-block parity flip:
    each engine gets exactly 4096 of the 8192 columns of every (x, m)
    block, finely interleaved so neither consumer phase-locks idle."""
    out = {}
    blk = 0
    for x_i in range(len(X_NAMES)):
        for m in range(MB):
            for g in range(NG):
                out[(x_i, m, g)] = "A" if g % 2 == 0 else "D"
            blk += 1
    return out


ASSIGN = _assignments()

# per-row count baseline: row in (x, m) accumulates only its ACT groups
ACT_COLS_ROW = {
    x: np.tile(
        np.repeat(
            [
                sum(
                    GSIZES[g]
                    for g in range(NG)
                    if ASSIGN[(x_i, m, g)] == "A"
                )
                for m in range(MB)
            ],
            128,
        ),
        NCORES,
    )
    for x_i, x in enumerate(X_NAMES)
}

_built = None


def _build_nc():
    import concourse.bacc as bacc
    import concourse.bass_isa as bass_isa
    import concourse.mybir as mybir
    import concourse.tile as tile

    F32 = mybir.dt.float32
    F32R = mybir.dt.float32r
    BF16 = mybir.dt.bfloat16
    F8 = mybir.dt.float8e4
    AF = mybir.ActivationFunctionType
    AX = mybir.AxisListType
    DR = mybir.MatmulPerfMode.DoubleRow

    nc = bacc.Bacc("TRN2", target_bir_lowering=False, debug=False)

    d_in = {}
    d_in["ce8"] = nc.dram_tensor("ce8", [128, KCH, BB], F8, kind="ExternalInput")
    for x in X_NAMES:
        d_in["q8" + x] = nc.dram_tensor(
            "q8" + x, [128, KCH * RPC], F8, kind="ExternalInput"
        )
    d_in["nd"] = nc.dram_tensor("nd", [128, 3 * MB], F32, kind="ExternalInput")
    d_in["xin"] = nc.dram_tensor(
        "xin", [128, 4 * KCH * RPC], BF16, kind="ExternalInput"
    )

    d_out = {}
    for nm in Q_NAMES:
        d_out[nm] = nc.dram_tensor(nm, [128, MB], F32, kind="ExternalOutput")
    for x in X_NAMES:
        for pre in ["mx_", "cnt_"]:
            d_out[pre + x] = nc.dram_tensor(
                pre + x, [128, MB], F32, kind="ExternalOutput"
            )

    with tile.TileContext(nc) as tc:
        with (
            tc.tile_pool(name="per", bufs=1) as per,
            tc.tile_pool(name="sc", bufs=2) as sc,
            tc.tile_pool(name="pr", bufs=6) as pr,
            tc.tile_pool(name="pb", bufs=1, space="PSUM") as pb,
        ):
            _rot = [0]

            def next_pt(sz=2048):
                i = _rot[0] % 2
                _rot[0] += 1
                return pb.tile([128, 2048], F32, name=f"pt{i}")
            # ---- persistent SBUF tiles + input DMAs (few, prioritized:
            # first block's weights, then ce8 chunks, then the rest) ----
            xq8 = {}
            negd = {}
            ndall = per.tile([128, 3, MB], F32, name="ndall")
            for x_i, x in enumerate(X_NAMES):
                xq8[x] = per.tile([128, KCH, RPC], F8, name=f"q8{x}")
                negd[x] = ndall[:, x_i, :]
            nc.sync.dma_start(xq8["v"][:], d_in["q8v"][:])
            nc.sync.dma_start(ndall[:], d_in["nd"][:])
            ce8g = []
            for g in range(NG):
                t = per.tile([128, KCH, GSIZES[g]], F8, name=f"ce8g{g}")
                ce8g.append(t)
                nc.sync.dma_start(
                    t[:], d_in["ce8"][:, :, GOFF[g] : GOFF[g] + GSIZES[g]]
                )
            for x in X_NAMES[1:]:
                nc.sync.dma_start(xq8[x][:], d_in["q8" + x][:])
            xin_all = per.tile([128, 4, KCH, RPC], BF16, name="xinall")
            nc.sync.dma_start(xin_all[:], d_in["xin"][:])
            xin = {
                nm: xin_all[:, i, :, :]
                for i, nm in enumerate(["vt", "gvt", "tpt", "cet"])
            }

            ones_f = per.tile([128, 2], F32, name="ones_f")
            nc.gpsimd.memset(ones_f[:], 1.0)
            ones = per.tile([128, 2], BF16, name="ones")
            nc.vector.tensor_copy(ones[:], ones_f[:])

            mxsl = {}
            cntsl = {}
            mxr = {}
            cntr = {}
            for x in X_NAMES:
                mxsl[x] = per.tile([128, MB, NG], F32, name=f"mxsl_{x}")
                cntsl[x] = per.tile([128, MB, NG], F32, name=f"cntsl_{x}")
                mxr[x] = per.tile([128, MB], F32, name=f"mxr_{x}")
                cntr[x] = per.tile([128, MB], F32, name=f"cntr_{x}")
                nc.gpsimd.memset(mxsl[x][:], NEG_INF)
                nc.gpsimd.memset(cntsl[x][:], 0.0)

            # ---- loss colsums: products mid-run (idle GPSIMD); the per-m
            # ones-matmul column sums run at the END, inside the consumer
            # drain window when the PE is otherwise idle ----
            els_done = []

            def emit_loss_psq():
                for batch in range(2):
                    pt = next_pt(2048)
                    for j in range(3):
                        name, elems = els_done[3 * batch + j]
                        base = j * MB * 2
                        for m in range(MB):
                            ms = slice(m * 128, (m + 1) * 128)
                            for k in range(KCH):
                                nc.tensor.matmul(
                                    pt[:, base + m * 2 : base + m * 2 + 2],
                                    elems[:, k, ms],
                                    ones[:],
                                    start=(k == 0),
                                    stop=(k == KCH - 1),
                                )
                    for j in range(3):
                        name, elems = els_done[3 * batch + j]
                        base = j * MB * 2
                        qcol = sc.tile([128, MB], F32, name="qcol")
                        nc.vector.tensor_scalar_add(
                            qcol[:], pt[:, base : base + MB * 2 : 2], 0.0
                        )
                        nc.sync.dma_start(d_out[name][:], qcol[:])

            def mk_sq(nm):
                sqq = pr.tile([128, KCH, RPC], BF16, name="els")
                nc.gpsimd.tensor_mul(sqq[:], xin[nm][:], xin[nm][:])
                return sqq

            def mk_mul(a, b):
                prod = pr.tile([128, KCH, RPC], BF16, name="els")
                nc.gpsimd.tensor_mul(prod[:], xin[a][:], xin[b][:])
                return prod

            loss_items = [
                ("q_dot_vf", lambda: mk_mul("vt", "gvt")),
                ("q_ss_v", lambda: mk_sq("vt")),
                ("q_ss_gv", lambda: mk_sq("gvt")),
                ("q_dot_tc", lambda: mk_mul("tpt", "cet")),
                ("q_ss_tp", lambda: mk_sq("tpt")),
                ("q_ss_ce", lambda: mk_sq("cet")),
            ]

            def emit_sim_group(x_i, x, m, g, slots):
                ms = slice(m * 128, (m + 1) * 128)
                sz = GSIZES[g]
                pt = next_pt(sz)
                for j in range(sz // MMN):
                    c0 = j * MMN
                    nc.tensor.matmul(
                        pt[:, j * MMN : (j + 1) * MMN],
                        xq8[x][:, :, ms],
                        ce8g[g][:, :, c0 : c0 + MMN],
                        start=True,
                        stop=True,
                        perf_mode=DR,
                    )
                a = ASSIGN[(x_i, m, g)]
                if a == "A":
                    dump = sc.tile([128, 2048], BF16, name="dump")
                    nc.scalar.activation(
                        dump[:, 0:sz],
                        pt[:, 0:sz],
                        AF.Sign,
                        bias=negd[x][:, m : m + 1],
                        accum_out=cntsl[x][:, m, slots[0] : slots[0] + 1],
                    )
                    slots[0] += 1
                else:
                    nc.vector.reduce_max(
                        mxsl[x][:, m, slots[1] : slots[1] + 1],
                        pt[:, 0:sz],
                        axis=AX.X,
                    )
                    slots[1] += 1

            # ---- sims (x,m)-major with a chunk-arrival-ordered prologue;
            # loss colsums every other block (GPSIMD products);
            # per-x folds as soon as that x's blocks finish ----
            slots = {(x, m): [0, 0] for x in X_NAMES for m in range(MB)}
            li = 0
            blk = 0
            for x_i, x in enumerate(X_NAMES):
                for m in range(MB):
                    if x_i == 0 and m < 4:
                        # prologue: first four blocks in two half passes so
                        # the first sims only need the first ce8 chunks
                        for g in range(NG // 2):
                            emit_sim_group(x_i, x, m, g, slots[(x, m)])
                        if m == 3:
                            for m2 in range(4):
                                for g in range(NG // 2, NG):
                                    emit_sim_group(
                                        x_i, x, m2, g, slots[(x, m2)]
                                    )
                    else:
                        for g in range(NG):
                            emit_sim_group(x_i, x, m, g, slots[(x, m)])
                    blk += 1
                    if blk % 2 == 1 and blk >= 5 and li < len(loss_items):
                        nm_, fn = loss_items[li]
                        li += 1
                        els_done.append((nm_, fn()))
                nc.vector.reduce_max(mxr[x][:], mxsl[x][:], axis=AX.X)
                nc.vector.reduce_sum(cntr[x][:], cntsl[x][:], axis=AX.X)
                nc.sync.dma_start(d_out["mx_" + x][:], mxr[x][:])
                nc.sync.dma_start(d_out["cnt_" + x][:], cntr[x][:])
            emit_loss_psq()

    _dedup_ldweights(nc)
    nc.compile()
    return nc


def _dedup_ldweights(nc):
    """Remove back-to-back duplicate LDWEIGHTS within a matmul group.

    Safe only when the duplicate loads identical weights AND its matmul
    writes the same PSUM allocation as the previous one (same group ->
    identical wait set) AND the LDW carries no semaphore updates. Any
    waits on the duplicate are merged into the following matmul.
    """
    import concourse.mybir as mybir

    for blk in nc.main_func.blocks:
        insts = list(blk.instructions)
        keep = []
        prev_sig = None
        prev_out = None
        removed = 0
        for idx, inst in enumerate(insts):
            tn = type(inst).__name__
            if tn == "InstLdweights":
                w = inst.ins[0]
                sig = (
                    str(w.memref),
                    int(w.offset),
                    str(w.ap),
                    str(w.dtype),
                    str(getattr(inst, "perf_mode", None)),
                )
                nxt = insts[idx + 1] if idx + 1 < len(insts) else None
                nxt_out = (
                    str(nxt.outs[0].memref)
                    if nxt is not None
                    and type(nxt).__name__ == "InstMatmult"
                    and nxt.outs
                    else None
                )
                si = inst.sync_info
                no_updates = si is None or len(si.on_update) == 0
                if (
                    sig == prev_sig
                    and nxt_out is not None
                    and nxt_out == prev_out
                    and no_updates
                ):
                    if si is not None and len(si.on_wait) > 0:
                        msi = nxt.sync_info
                        if msi is None:
                            nxt.sync_info = mybir.SyncInfo(
                                on_wait=list(si.on_wait), on_update=[]
                            )
                        else:
                            nxt.sync_info = mybir.SyncInfo(
                                on_wait=list(msi.on_wait) + list(si.on_wait),
                                on_update=list(msi.on_update),
                            )
                    removed += 1
                    continue
                prev_sig = sig
                prev_out = nxt_out
                keep.append(inst)
                continue
            if tn == "InstMatmult":
                if inst.outs:
                    prev_out_now = str(inst.outs[0].memref)
                    if prev_out is not None and prev_out_now != prev_out:
                        prev_sig = None
                        prev_out = None
            elif getattr(inst, "engine", None) == mybir.EngineType.PE:
                prev_sig = None
                prev_out = None
            keep.append(inst)
        if removed:
            while len(blk.instructions) > 0:
                blk.instructions.pop()
            for inst in keep:
                blk.instructions.append(inst)


def _get_nc():
    global _built
    if _built is None:
        _built = _build_nc()
    return _built


def _img(a_t):
    # [256, N] (k-major rows) -> SBUF image [128, 2*N] (k-chunk pairs per row)
    n = a_t.shape[1]
    return np.ascontiguousarray(
        a_t.reshape(KCH, 128, n).transpose(1, 0, 2).reshape(128, KCH * n)
    )


_HOST_DPLUS = None


def _make_in_maps(inputs):
    global _HOST_DPLUS
    import ml_dtypes

    BF = ml_dtypes.bfloat16
    E4 = ml_dtypes.float8_e4m3

    vp = np.asarray(inputs["vis_pred"], dtype=np.float32)
    tp = np.asarray(inputs["text_pred"], dtype=np.float32)
    gv = np.asarray(inputs["vis_feats_proj"], dtype=np.float32)
    ce = np.asarray(inputs["caption_emb"], dtype=np.float32)
    nv = np.asarray(inputs["vis_feats_proj_narr"], dtype=np.float32)

    # normalized + fp8 caption side (shared by all cores)
    ce_bf = ce.astype(BF).astype(np.float32)
    s = 1.0 / np.maximum(np.sqrt((ce_bf.astype(np.float64) ** 2).sum(-1)), EPS)
    t8 = (ce_bf * s[:, None].astype(np.float32)).astype(E4)  # [BB, DIM]
    q8 = {"v": vp.astype(E4), "gv": gv.astype(E4), "nv": nv.astype(E4)}

    # exact diagonal dots from the fp8 bytes
    t8_64 = t8.astype(np.float64)
    d_full = {x: (q8[x].astype(np.float64) * t8_64).sum(-1) for x in X_NAMES}
    _HOST_DPLUS = {x: d_full[x] + TOL for x in X_NAMES}

    bf16 = {
        "vt": vp.astype(BF),
        "gvt": gv.astype(BF),
        "tpt": tp.astype(BF),
        "cet": ce.astype(BF),
    }

    in_maps = []
    for c in range(NCORES):
        sl = slice(c * RPC, (c + 1) * RPC)
        m = {
            "ce8": _img(
                np.ascontiguousarray(np.roll(t8.T, -c * RPC, axis=1))
            ).reshape(128, KCH, BB)
        }
        nds = []
        for x in X_NAMES:
            m["q8" + x] = _img(np.ascontiguousarray(q8[x][sl].T))
            nds.append(
                -(d_full[x][sl] + TOL).astype(np.float32).reshape(MB, 128).T
            )
        m["nd"] = np.ascontiguousarray(np.concatenate(nds, axis=1))
        m["xin"] = np.ascontiguousarray(
            np.concatenate(
                [
                    _img(np.ascontiguousarray(bf16[nm][sl].T))
                    for nm in ["vt", "gvt", "tpt", "cet"]
                ],
                axis=1,
            )
        )
        in_maps.append(m)
    return in_maps


def _run(in_maps, **kwargs):
    from concourse.bass_utils import run_bass_kernel_spmd

    return run_bass_kernel_spmd(
        _get_nc(), in_maps, core_ids=list(range(NCORES)), **kwargs
    )


def _unpack(results, name):
    # [128, MB] per core, local row = m*128 + p -> concat to [BB]
    return np.concatenate([r[name].T.reshape(-1) for r in results])


def _combine(results):
    q = {nm: _unpack(results, nm) for nm in Q_NAMES}

    def cos(dot, ssa, ssb):
        na = np.maximum(np.sqrt(ssa), EPS)
        nb = np.maximum(np.sqrt(ssb), EPS)
        return (dot / (na * nb)).astype(np.float32)

    cos_v = cos(q["q_dot_vf"], q["q_ss_v"], q["q_ss_gv"])
    cos_t = cos(q["q_dot_tc"], q["q_ss_tp"], q["q_ss_ce"])
    vis_loss = np.float32(np.mean((np.float32(1.0) - cos_v)))
    text_loss = np.float32(np.mean((np.float32(1.0) - cos_t)))
    loss = np.float32(vis_loss + text_loss)

    accs = {}
    for x in X_NAMES:
        dp = _HOST_DPLUS[x]
        mx = _unpack(results, "mx_" + x)
        cnt = _unpack(results, "cnt_" + x)
        n_greater = (ACT_COLS_ROW[x] + cnt) / 2.0
        ok = (dp >= mx) & (n_greater < 0.5)
        accs[x] = np.float32(100.0 * np.float32(ok.sum()) / np.float32(BB))

    return np.stack(
        [
            vis_loss,
            text_loss,
            loss,
            accs["gv"],
            accs["v"],
            accs["gv"],
            accs["v"],
            accs["nv"],
            accs["nv"],
        ]
    ).astype(np.float32)


def kernel(**inputs):
    return _combine(_run(_make_in_maps(inputs)).results)


# revision 50
# speedup vs baseline: 1.1952x; 1.1952x over previous
"""Trainium2 Bass kernel for nn_EstLossSepEmb (contrastive eval loss_fn).

Strategy (data-parallel over the batch dim, 8 cores, 1024 rows each):
  - Host prep (layout/dtype only): normalize caption_emb rows and cast to
    fp8e4; cast the three query tensors to fp8e4 and the four loss-side
    tensors to bf16; build per-core SBUF-image layouts ([128, 2, N]:
    k-chunk pairs per partition) with caption_emb rolled so each core's
    own 1024 text rows come first; precompute the per-row diagonal bias
    -(d_i+TOL) from the same fp8 bytes (exact in fp64; the PSUM-fp32 sim
    value of the diagonal deviates by ~1e-6 << TOL).
  - Device (all the FLOPs):
      * 3 big sims as fp8e4 DoubleRow matmuls [1024,256]x[256,8192]:
        K=256 contracted in ONE pass at 2x bf16 rate, 512 out-cols per
        matmul (s3d3 ISA max), weights stationary per (x, m-block) with
        duplicate LDWEIGHTS removed by a custom post-schedule pass.
        fp8 sim noise ~0.004 cos vs measured min decision margin 0.030.
      * per row decide "argmax == i" as (d+TOL >= rowmax) AND
        count(sim > d+TOL) == 0. PSUM groups of 1536/1536/1024 columns
        rotate through three single-buffered slots (3+3+2 banks =
        pipeline depth 3); each group is consumed whole by ACT
        Sign(bias=-(d+TOL))+accum (count) or DVE reduce_max (rowmax) --
        the only two PSUM-capable consumers -- strictly alternating so
        both run concurrently.
      * rowwise-cos loss ingredients (dots + sumsqs of bf16 inputs):
        GPSIMD elementwise products (otherwise-idle engine) + per-m-block
        ones-matmul column sums into the 1024-slot, in [128, MB] layout.
      * emission order: (x, m)-major with a chunk-arrival prologue for
        the first four blocks; loss colsums every other block; per-x
        slot folds + output DMAs as soon as that x finishes.
  - Host combine: means / cos / counts -> the 9-vector output.
"""

import os

import numpy as np

BB = 8192
DIM = 256
NCORES = 8
RPC = BB // NCORES  # rows per core = 1024
MB = RPC // 128  # m blocks per core = 8
KCH = DIM // 128  # 2 k-chunks
# mixed-size PSUM groups: three rotating single-buffer tiles (3+3+2 banks)
# give pipeline depth 3, decoupling the PE from both consumers
GSIZES = [1536, 1536, 1024, 1536, 1536, 1024]
GOFF = [0, 1536, 3072, 4096, 5632, 7168]
NG = len(GSIZES)  # 6 groups per (x, m)
MMN = int(os.environ.get("K_MMN", "512"))  # out cols per DoubleRow matmul
TOL = 1e-4
EPS = 1e-8
NEG_INF = -3.0e38

# consumer-engine shares for the 96 (x, m, g) sim groups (ACT Sign+accum
# count vs DVE reduce_max; GPSIMD cannot access PSUM on TRN2)
W_ACT = float(os.environ.get("K_WA", "0.49"))
W_DVE = float(os.environ.get("K_WD", "0.51"))

Q_NAMES = ["q_dot_vf", "q_ss_v", "q_ss_gv", "q_dot_tc", "q_ss_tp", "q_ss_ce"]
X_NAMES = ["v", "gv", "nv"]


def _assignments():
    """Strict per-group engine alternation with per-block parity flip:
    each engine gets exactly 4096 of the 8192 columns of every (x, m)
    block, finely interleaved so neither consumer phase-locks idle."""
    out = {}
    blk = 0
    for x_i in range(len(X_NAMES)):
        for m in range(MB):
            for g in range(NG):
                out[(x_i, m, g)] = "A" if g % 2 == 0 else "D"
            blk += 1
    return out


ASSIGN = _assignments()

# per-row count baseline: row in (x, m) accumulates only its ACT groups
ACT_COLS_ROW = {
    x: np.tile(
        np.repeat(
            [
                sum(
                    GSIZES[g]
                    for g in range(NG)
                    if ASSIGN[(x_i, m, g)] == "A"
                )
                for m in range(MB)
            ],
            128,
        ),
        NCORES,
    )
    for x_i, x in enumerate(X_NAMES)
}

_built = None


def _build_nc():
    import concourse.bacc as bacc
    import concourse.bass_isa as bass_isa
    import concourse.mybir as mybir
    import concourse.tile as tile

    F32 = mybir.dt.float32
    F32R = mybir.dt.float32r
    BF16 = mybir.dt.bfloat16
    F8 = mybir.dt.float8e4
    AF = mybir.ActivationFunctionType
    AX = mybir.AxisListType
    DR = mybir.MatmulPerfMode.DoubleRow

    nc = bacc.Bacc("TRN2", target_bir_lowering=False, debug=False)

    d_in = {}
    d_in["ce8"] = nc.dram_tensor("ce8", [128, KCH, BB], F8, kind="ExternalInput")
    for x in X_NAMES:
        d_in["q8" + x] = nc.dram_tensor(
            "q8" + x, [128, KCH * RPC], F8, kind="ExternalInput"
        )
    d_in["nd"] = nc.dram_tensor("nd", [128, 3 * MB], F32, kind="ExternalInput")
    d_in["xin"] = nc.dram_tensor(
        "xin", [128, 4 * KCH * RPC], BF16, kind="ExternalInput"
    )

    d_out = {}
    for nm in Q_NAMES:
        d_out[nm] = nc.dram_tensor(nm, [128, MB], F32, kind="ExternalOutput")
    for x in X_NAMES:
        for pre in ["mx_", "cnt_"]:
            d_out[pre + x] = nc.dram_tensor(
                pre + x, [128, MB], F32, kind="ExternalOutput"
            )

    with tile.TileContext(nc) as tc:
        with (
            tc.tile_pool(name="per", bufs=1) as per,
            tc.tile_pool(name="sc", bufs=2) as sc,
            tc.tile_pool(name="pr", bufs=6) as pr,
            tc.tile_pool(name="pb", bufs=1, space="PSUM") as pb,
        ):
            _rot = [0]

            def next_pt(sz=1024):
                if sz == 1536:
                    i = _rot[0] % 2
                    _rot[0] += 1
                    return pb.tile([128, 1536], F32, name=f"pt{i}")
                return pb.tile([128, 1024], F32, name="ptC")
            # ---- persistent SBUF tiles + input DMAs (few, prioritized:
            # first block's weights, then ce8 chunks, then the rest) ----
            xq8 = {}
            negd = {}
            ndall = per.tile([128, 3, MB], F32, name="ndall")
            for x_i, x in enumerate(X_NAMES):
                xq8[x] = per.tile([128, KCH, RPC], F8, name=f"q8{x}")
                negd[x] = ndall[:, x_i, :]
            nc.sync.dma_start(xq8["v"][:], d_in["q8v"][:])
            nc.sync.dma_start(ndall[:], d_in["nd"][:])
            ce8g = []
            for g in range(NG):
                t = per.tile([128, KCH, GSIZES[g]], F8, name=f"ce8g{g}")
                ce8g.append(t)
                nc.sync.dma_start(
                    t[:], d_in["ce8"][:, :, GOFF[g] : GOFF[g] + GSIZES[g]]
                )
            for x in X_NAMES[1:]:
                nc.sync.dma_start(xq8[x][:], d_in["q8" + x][:])
            xin_all = per.tile([128, 4, KCH, RPC], BF16, name="xinall")
            nc.sync.dma_start(xin_all[:], d_in["xin"][:])
            xin = {
                nm: xin_all[:, i, :, :]
                for i, nm in enumerate(["vt", "gvt", "tpt", "cet"])
            }

            ones_f = per.tile([128, 2], F32, name="ones_f")
            nc.gpsimd.memset(ones_f[:], 1.0)
            ones = per.tile([128, 2], BF16, name="ones")
            nc.vector.tensor_copy(ones[:], ones_f[:])

            mxsl = {}
            cntsl = {}
            mxr = {}
            cntr = {}
            for x in X_NAMES:
                mxsl[x] = per.tile([128, MB, NG], F32, name=f"mxsl_{x}")
                cntsl[x] = per.tile([128, MB, NG], F32, name=f"cntsl_{x}")
                mxr[x] = per.tile([128, MB], F32, name=f"mxr_{x}")
                cntr[x] = per.tile([128, MB], F32, name=f"cntr_{x}")
                nc.gpsimd.memset(mxsl[x][:], NEG_INF)
                nc.gpsimd.memset(cntsl[x][:], 0.0)

            # ---- loss colsums: products mid-run (idle GPSIMD); the per-m
            # ones-matmul column sums run at the END, inside the consumer
            # drain window when the PE is otherwise idle ----
            els_done = []

            def emit_loss_psq():
                for batch in range(2):
                    pt = next_pt(1024 if batch == 0 else 1536)
                    for j in range(3):
                        name, elems = els_done[3 * batch + j]
                        base = j * MB * 2
                        for m in range(MB):
                            ms = slice(m * 128, (m + 1) * 128)
                            for k in range(KCH):
                                nc.tensor.matmul(
                                    pt[:, base + m * 2 : base + m * 2 + 2],
                                    elems[:, k, ms],
                                    ones[:],
                                    start=(k == 0),
                                    stop=(k == KCH - 1),
                                )
                    for j in range(3):
                        name, elems = els_done[3 * batch + j]
                        base = j * MB * 2
                        qcol = sc.tile([128, MB], F32, name="qcol")
                        nc.vector.tensor_scalar_add(
                            qcol[:], pt[:, base : base + MB * 2 : 2], 0.0
                        )
                        nc.sync.dma_start(d_out[name][:], qcol[:])

            def mk_sq(nm):
                sqq = pr.tile([128, KCH, RPC], BF16, name="els")
                nc.gpsimd.tensor_mul(sqq[:], xin[nm][:], xin[nm][:])
                return sqq

            def mk_mul(a, b):
                prod = pr.tile([128, KCH, RPC], BF16, name="els")
                nc.gpsimd.tensor_mul(prod[:], xin[a][:], xin[b][:])
                return prod

            loss_items = [
                ("q_dot_vf", lambda: mk_mul("vt", "gvt")),
                ("q_ss_v", lambda: mk_sq("vt")),
                ("q_ss_gv", lambda: mk_sq("gvt")),
                ("q_dot_tc", lambda: mk_mul("tpt", "cet")),
                ("q_ss_tp", lambda: mk_sq("tpt")),
                ("q_ss_ce", lambda: mk_sq("cet")),
            ]

            def emit_sim_group(x_i, x, m, g, slots):
                ms = slice(m * 128, (m + 1) * 128)
                sz = GSIZES[g]
                pt = next_pt(sz)
                for j in range(sz // MMN):
                    c0 = j * MMN
                    nc.tensor.matmul(
                        pt[:, j * MMN : (j + 1) * MMN],
                        xq8[x][:, :, ms],
                        ce8g[g][:, :, c0 : c0 + MMN],
                        start=True,
                        stop=True,
                        perf_mode=DR,
                    )
                a = ASSIGN[(x_i, m, g)]
                if a == "A":
                    dump = sc.tile([128, 1536], BF16, name="dump")
                    nc.scalar.activation(
                        dump[:, 0:sz],
                        pt[:, 0:sz],
                        AF.Sign,
                        bias=negd[x][:, m : m + 1],
                        accum_out=cntsl[x][:, m, slots[0] : slots[0] + 1],
                    )
                    slots[0] += 1
                else:
                    nc.vector.reduce_max(
                        mxsl[x][:, m, slots[1] : slots[1] + 1],
                        pt[:, 0:sz],
                        axis=AX.X,
                    )
                    slots[1] += 1

            # ---- sims (x,m)-major with a chunk-arrival-ordered prologue;
            # loss colsums every other block (GPSIMD products);
            # per-x folds as soon as that x's blocks finish ----
            slots = {(x, m): [0, 0] for x in X_NAMES for m in range(MB)}
            li = 0
            blk = 0
            for x_i, x in enumerate(X_NAMES):
                for m in range(MB):
                    if x_i == 0 and m < 4:
                        # prologue: first four blocks in two half passes so
                        # the first sims only need the first ce8 chunks
                        for g in range(NG // 2):
                            emit_sim_group(x_i, x, m, g, slots[(x, m)])
                        if m == 3:
                            for m2 in range(4):
                                for g in range(NG // 2, NG):
                                    emit_sim_group(
                                        x_i, x, m2, g, slots[(x, m2)]
                                    )
                    else:
                        for g in range(NG):
                            emit_sim_group(x_i, x, m, g, slots[(x, m)])
                    blk += 1
                    if blk % 2 == 1 and blk >= 5 and li < len(loss_items):
                        nm_, fn = loss_items[li]
                        li += 1
                        els_done.append((nm_, fn()))
                nc.vector.reduce_max(mxr[x][:], mxsl[x][:], axis=AX.X)
                nc.vector.reduce_sum(cntr[x][:], cntsl[x][:], axis=AX.X)
                nc.sync.dma_start(d_out["mx_" + x][:], mxr[x][:])
                nc.sync.dma_start(d_out["cnt_" + x][:], cntr[x][:])
            emit_loss_psq()

    _dedup_ldweights(nc)
    nc.compile()
    return nc


def _dedup_ldweights(nc):
    """Remove back-to-back duplicate LDWEIGHTS within a matmul group.

    Safe only when the duplicate loads identical weights AND its matmul
    writes the same PSUM allocation as the previous one (same group ->
    identical wait set) AND the LDW carries no semaphore updates. Any
    waits on the duplicate are merged into the following matmul.
    """
    import concourse.mybir as mybir

    for blk in nc.main_func.blocks:
        insts = list(blk.instructions)
        keep = []
        prev_sig = None
        prev_out = None
        removed = 0
        for idx, inst in enumerate(insts):
            tn = type(inst).__name__
            if tn == "InstLdweights":
                w = inst.ins[0]
                sig = (
                    str(w.memref),
                    int(w.offset),
                    str(w.ap),
                    str(w.dtype),
                    str(getattr(inst, "perf_mode", None)),
                )
                nxt = insts[idx + 1] if idx + 1 < len(insts) else None
                nxt_out = (
                    str(nxt.outs[0].memref)
                    if nxt is not None
                    and type(nxt).__name__ == "InstMatmult"
                    and nxt.outs
                    else None
                )
                si = inst.sync_info
                no_updates = si is None or len(si.on_update) == 0
                if (
                    sig == prev_sig
                    and nxt_out is not None
                    and nxt_out == prev_out
                    and no_updates
                ):
                    if si is not None and len(si.on_wait) > 0:
                        msi = nxt.sync_info
                        if msi is None:
                            nxt.sync_info = mybir.SyncInfo(
                                on_wait=list(si.on_wait), on_update=[]
                            )
                        else:
                            nxt.sync_info = mybir.SyncInfo(
                                on_wait=list(msi.on_wait) + list(si.on_wait),
                                on_update=list(msi.on_update),
                            )
                    removed += 1
                    continue
                prev_sig = sig
                prev_out = nxt_out
                keep.append(inst)
                continue
            if tn == "InstMatmult":
                if inst.outs:
                    prev_out_now = str(inst.outs[0].memref)
                    if prev_out is not None and prev_out_now != prev_out:
                        prev_sig = None
                        prev_out = None
            elif getattr(inst, "engine", None) == mybir.EngineType.PE:
                prev_sig = None
                prev_out = None
            keep.append(inst)
        if removed:
            while len(blk.instructions) > 0:
                blk.instructions.pop()
            for inst in keep:
                blk.instructions.append(inst)


def _get_nc():
    global _built
    if _built is None:
        _built = _build_nc()
    return _built


def _img(a_t):
    # [256, N] (k-major rows) -> SBUF image [128, 2*N] (k-chunk pairs per row)
    n = a_t.shape[1]
    return np.ascontiguousarray(
        a_t.reshape(KCH, 128, n).transpose(1, 0, 2).reshape(128, KCH * n)
    )


_HOST_DPLUS = None


def _make_in_maps(inputs):
    global _HOST_DPLUS
    import ml_dtypes

    BF = ml_dtypes.bfloat16
    E4 = ml_dtypes.float8_e4m3

    vp = np.asarray(inputs["vis_pred"], dtype=np.float32)
    tp = np.asarray(inputs["text_pred"], dtype=np.float32)
    gv = np.asarray(inputs["vis_feats_proj"], dtype=np.float32)
    ce = np.asarray(inputs["caption_emb"], dtype=np.float32)
    nv = np.asarray(inputs["vis_feats_proj_narr"], dtype=np.float32)

    # normalized + fp8 caption side (shared by all cores)
    ce_bf = ce.astype(BF).astype(np.float32)
    s = 1.0 / np.maximum(np.sqrt((ce_bf.astype(np.float64) ** 2).sum(-1)), EPS)
    t8 = (ce_bf * s[:, None].astype(np.float32)).astype(E4)  # [BB, DIM]
    q8 = {"v": vp.astype(E4), "gv": gv.astype(E4), "nv": nv.astype(E4)}

    # exact diagonal dots from the fp8 bytes
    t8_64 = t8.astype(np.float64)
    d_full = {x: (q8[x].astype(np.float64) * t8_64).sum(-1) for x in X_NAMES}
    _HOST_DPLUS = {x: d_full[x] + TOL for x in X_NAMES}

    bf16 = {
        "vt": vp.astype(BF),
        "gvt": gv.astype(BF),
        "tpt": tp.astype(BF),
        "cet": ce.astype(BF),
    }

    in_maps = []
    for c in range(NCORES):
        sl = slice(c * RPC, (c + 1) * RPC)
        m = {
            "ce8": _img(
                np.ascontiguousarray(np.roll(t8.T, -c * RPC, axis=1))
            ).reshape(128, KCH, BB)
        }
        nds = []
        for x in X_NAMES:
            m["q8" + x] = _img(np.ascontiguousarray(q8[x][sl].T))
            nds.append(
                -(d_full[x][sl] + TOL).astype(np.float32).reshape(MB, 128).T
            )
        m["nd"] = np.ascontiguousarray(np.concatenate(nds, axis=1))
        m["xin"] = np.ascontiguousarray(
            np.concatenate(
                [
                    _img(np.ascontiguousarray(bf16[nm][sl].T))
                    for nm in ["vt", "gvt", "tpt", "cet"]
                ],
                axis=1,
            )
        )
        in_maps.append(m)
    return in_maps


def _run(in_maps, **kwargs):
    from concourse.bass_utils import run_bass_kernel_spmd

    return run_bass_kernel_spmd(
        _get_nc(), in_maps, core_ids=list(range(NCORES)), **kwargs
    )


def _unpack(results, name):
    # [128, MB] per core, local row = m*128 + p -> concat to [BB]
    return np.concatenate([r[name].T.reshape(-1) for r in results])


def _combine(results):
    q = {nm: _unpack(results, nm) for nm in Q_NAMES}

    def cos(dot, ssa, ssb):
        na = np.maximum(np.sqrt(ssa), EPS)
        nb = np.maximum(np.sqrt(ssb), EPS)
        return (dot / (na * nb)).astype(np.float32)

    cos_v = cos(q["q_dot_vf"], q["q_ss_v"], q["q_ss_gv"])
    cos_t = cos(q["q_dot_tc"], q["q_ss_tp"], q["q_ss_ce"])
    vis_loss = np.float32(np.mean((np.float32(1.0) - cos_v)))
    text_loss = np.float32(np.mean((np.float32(1.0) - cos_t)))
    loss = np.float32(vis_loss + text_loss)

    accs = {}
    for x in X_NAMES:
        dp = _HOST_DPLUS[x]
        mx = _unpack(results, "mx_" + x)
        cnt = _unpack(results, "cnt_" + x)
        n_greater = (ACT_COLS_ROW[x] + cnt) / 2.0
        ok = (dp >= mx) & (n_greater < 0.5)
        accs[x] = np.float32(100.0 * np.float32(ok.sum()) / np.float32(BB))

    return np.stack(
        [
            vis_loss,
            text_loss,
            loss,
            accs["gv"],
            accs["v"],
            accs["gv"],
            accs["v"],
            accs["nv"],
            accs["nv"],
        ]
    ).astype(np.float32)


def kernel(**inputs):
    return _combine(_run(_make_in_maps(inputs)).results)
